# revision 1
# baseline (speedup 1.0000x reference)
"""Trainium2 Bass kernel for the triplet exp-distance loss.

loss = mean_i[ D_ap*(D_ap - v_ap)^2 + D_an*(D_an - v_an)^2 ]
  D_xx = exp(-triplets_dis[batch_index][:, k])
  v_xx = exp(-||a - x||_2)

Strategy: pure data parallel over 8 NeuronCores (65536 rows each).
Per core, SBUF partition p owns 512 contiguous rows; the shard streams
in 16 chunks of [128 part x 32 rows x 128 dim] f32 via 2MB HWDGE DMAs
(16KB contiguous per partition).  Per chunk: DVE computes diff
(f32 -> bf16), ACT squares in place (bf16), DVE tree-adds + reduces to
per-row squared norms.  A single batched tail does sqrt / exp and the
weighted squared error, accumulating into a [128, 2] partial that the
host sums across cores.
"""

import numpy as np

import concourse.bass as bass
import concourse.mybir as mb
import concourse.tile as tile
from concourse.bass_utils import run_bass_kernel_spmd

B = 524288
D = 128
M = 8                 # cores
S = B // M            # rows per core = 65536
P = 128               # SBUF partitions
RPP = S // P          # rows per partition = 512
C = 16                # rows per partition per chunk
NCH = RPP // C        # chunks
FD = C * D            # free-dim elements per chunk
IO_BUFS = 3           # input-tile double/triple buffering
GPSIMD_SUB = False    # offload one subtract per pair to GPSIMD

F32 = mb.dt.float32
BF16 = mb.dt.bfloat16


def _split_multiwaits(nc):
    """This walrus build accepts only one sync-wait per instruction.
    Hoist extra waits onto standalone single-wait InstEventSemaphore
    instructions inserted just before, on the same engine (semantically
    identical: the engine queue blocks on each in sequence)."""
    n_split = 0
    for f in nc.m.functions:
        for bb in f.blocks:
            insts = bb.instructions
            out = []
            changed = False
            for ins in insts:
                si = getattr(ins, "sync_info", None)
                if si is not None and si.on_wait is not None and len(si.on_wait) > 1:
                    waits = list(si.on_wait)
                    for k, w in enumerate(waits[:-1]):
                        ev = mb.InstEventSemaphore(
                            name=f"{ins.name}-wsplit{k}",
                            engine=ins.engine,
                            ins=[],
                            outs=[],
                            sync_info=mb.SyncInfo(on_wait=[w], on_update=[]),
                        )
                        out.append(ev)
                        n_split += 1
                    si.on_wait.clear()
                    si.on_wait.append(waits[-1])
                    changed = True
                out.append(ins)
            if changed:
                bb.instructions = out
    return n_split


def _build():
    nc = bass.Bass(trn_type="TRN2", name="triplet_loss")
    a = nc.dram_tensor("a", [S, D], F32, kind="ExternalInput")
    p = nc.dram_tensor("p", [S, D], F32, kind="ExternalInput")
    n = nc.dram_tensor("n", [S, D], F32, kind="ExternalInput")
    td = nc.dram_tensor("td", [S, 2], F32, kind="ExternalInput")
    out = nc.dram_tensor("out", [P, 4], F32, kind="ExternalOutput")

    # Partition p owns rows [p*RPP, (p+1)*RPP): contiguous per-partition
    # DRAM runs -> ideal DMA descriptors (16KB contiguous each).
    av = a.rearrange("(p n) d -> p (n d)", p=P)    # [128, RPP*D]
    pv = p.rearrange("(p n) d -> p (n d)", p=P)
    nv = n.rearrange("(p n) d -> p (n d)", p=P)
    tdv = td.rearrange("(p n) t -> p n t", p=P)    # [128, RPP, 2]

    with tile.TileContext(nc) as tc:
        with tc.tile_pool(name="io", bufs=IO_BUFS) as io, \
             tc.tile_pool(name="wk", bufs=2) as wk, \
             tc.tile_pool(name="tl", bufs=2) as tl, \
             tc.tile_pool(name="res", bufs=1) as res:
            td_t = res.tile([P, RPP, 2], F32)
            nc.sync.dma_start(out=td_t, in_=tdv)

            n2 = {}
            n2["p"] = res.tile([P, RPP], F32, tag="n2p", name="n2p")
            n2["n"] = res.tile([P, RPP], F32, tag="n2n", name="n2n")

            for c in range(NCH):
                sl = slice(c * FD, (c + 1) * FD)
                at = io.tile([P, FD], F32, tag="a")
                nc.sync.dma_start(out=at, in_=av[:, sl])
                pt = io.tile([P, FD], F32, tag="p")
                nc.sync.dma_start(out=pt, in_=pv[:, sl])
                nt = io.tile([P, FD], F32, tag="n")
                nc.sync.dma_start(out=nt, in_=nv[:, sl])

                at3 = at.rearrange("p (c d) -> p c d", d=D)
                for key, ot in (("p", pt), ("n", nt)):
                    ot3 = ot.rearrange("p (c d) -> p c d", d=D)
                    df = wk.tile([P, C, D], BF16, tag="d" + key)
                    # split the two subtracts across DVE and GPSIMD so the
                    # 1x f32 passes don't pile onto one engine
                    sub_eng = nc.gpsimd if (GPSIMD_SUB and key == "p") else nc.vector
                    sub_eng.tensor_sub(out=df, in0=at3, in1=ot3)
                    # square in place on ACT (bf16, 1x)
                    nc.scalar.activation(
                        out=df, in_=df, func=mb.ActivationFunctionType.Square
                    )
                    # bf16 2x tree adds, then 1x reduce of the last quarter
                    h1 = wk.tile([P, C, D // 2], BF16, tag="h1" + key)
                    nc.vector.tensor_add(
                        out=h1, in0=df[:, :, 0 : D // 2], in1=df[:, :, D // 2 : D]
                    )
                    h2 = wk.tile([P, C, D // 4], BF16, tag="h2" + key)
                    nc.vector.tensor_add(
                        out=h2, in0=h1[:, :, 0 : D // 4], in1=h1[:, :, D // 4 : D // 2]
                    )
                    nc.vector.reduce_sum(
                        out=n2[key][:, c * C : (c + 1) * C],
                        in_=h2,
                        axis=mb.AxisListType.X,
                    )

            # ---- batched tail, in halves so half overlaps the chunk loop ----
            dex = res.tile([P, RPP, 2], F32)
            acc = res.tile([P, 2, 2], F32)   # [P, half, pair]

            def tail(h):
                rs = slice(h * (RPP // 2), (h + 1) * (RPP // 2))
                for key in ("p", "n"):
                    nv_ = n2[key][:, rs]
                    nc.scalar.activation(out=nv_, in_=nv_, func=mb.ActivationFunctionType.Sqrt)
                nc.scalar.activation(out=dex[:, rs, :], in_=td_t[:, rs, :],
                                     func=mb.ActivationFunctionType.Exp, scale=-1.0)
                for i, key in enumerate(("p", "n")):
                    nv_ = n2[key][:, rs]
                    nc.scalar.activation(out=nv_, in_=nv_, func=mb.ActivationFunctionType.Exp, scale=-1.0)
                    dcol = dex[:, rs, i]
                    t_ = tl.tile([P, RPP // 2], F32, tag="t")
                    nc.vector.tensor_sub(out=t_, in0=dcol, in1=nv_)
                    m_ = tl.tile([P, RPP // 2], F32, tag="m")
                    nc.vector.tensor_mul(out=m_, in0=dcol, in1=t_)
                    sc = tl.tile([P, RPP // 2], F32, tag="sc")
                    nc.vector.tensor_mul(out=sc, in0=m_, in1=t_)
                    nc.vector.reduce_sum(
                        out=acc[:, h, i : i + 1], in_=sc, axis=mb.AxisListType.X
                    )

            tail(0)
            tail(1)
            nc.sync.dma_start(out=out[:, :], in_=acc.rearrange('p h i -> p (h i)'))

    _split_multiwaits(nc)
    return nc


_CACHE = {}


def _get_nc():
    if "nc" not in _CACHE:
        _CACHE["nc"] = _build()
    return _CACHE["nc"]


def _run(inputs, **spmd_kwargs):
    a = np.asarray(inputs["embedding_a"], dtype=np.float32)
    p = np.asarray(inputs["embedding_p"], dtype=np.float32)
    n = np.asarray(inputs["embedding_n"], dtype=np.float32)
    tdis = np.asarray(inputs["triplets_dis"], dtype=np.float32)
    bidx = np.asarray(inputs["batch_index"])
    td = np.ascontiguousarray(tdis[bidx])

    in_maps = [
        {
            "a": a[i * S : (i + 1) * S],
            "p": p[i * S : (i + 1) * S],
            "n": n[i * S : (i + 1) * S],
            "td": td[i * S : (i + 1) * S],
        }
        for i in range(M)
    ]
    r = run_bass_kernel_spmd(_get_nc(), in_maps, core_ids=list(range(M)), **spmd_kwargs)
    total = sum(res["out"].astype(np.float64).sum() for res in r.results)
    return np.float32(total / B), r


def kernel(**inputs):
    loss, _ = _run(inputs)
    return loss



# revision 30
# speedup vs baseline: 1.7175x; 1.7175x over previous
"""Trainium2 Bass kernel for the triplet exp-distance loss.

loss = mean_i[ D_ap*(D_ap - v_ap)^2 + D_an*(D_an - v_an)^2 ]
  D_xx = exp(-triplets_dis[batch_index][:, k])
  v_xx = exp(-||a - x||_2)

Strategy: pure data parallel over 8 NeuronCores (65536 rows each).
Per core, SBUF partition p owns 512 contiguous rows.  The embeddings
stream in as bf16 via SWDGE cast-DMAs (f32 HBM -> bf16 SBUF), halving
the SBUF-side DMA traffic.  The subtraction itself rides the DMA: `-a`
(negated on the host) lands first and is duplicated on DVE (4x copy),
then `p`/`n` stream in with an accumulating-ADD DMA (CCE), producing
x-a directly in SBUF -- squaring makes the sign irrelevant.  Pool does nothing but
issue SWDGE DMAs (its SEQ is strictly in-order; compute there would
head-of-line-block the DMA stream).  Squares run in place (split
DVE/ACT by parity to balance), the tree-fold reduction and weighted-
error tail run on DVE, transcendentals on ACT.  The p-pair chain lags
its chunk by one and the loss tail by two, so no engine ever stalls
waiting on another's round-trip.  Chunk sizes taper at the end so
almost nothing trails the final DMA.
"""

import os

import numpy as np

import concourse.bass as bass
import concourse.mybir as mb
import concourse.tile as tile
from concourse.bass_utils import run_bass_kernel_spmd

B = 524288
D = 128
M = 8                 # cores
S = B // M            # rows per core = 65536
P = 128               # SBUF partitions
RPP = S // P          # rows per partition = 512
# rows-per-partition per chunk; tapered tail so the last chunk's
# compute pipeline after the final DMA is tiny
CHUNKS = [int(x) for x in os.environ.get("KCHUNKS", "64,64,64,64,64,64,64,32,24,8").split(",")]
assert sum(CHUNKS) == RPP
NCH = len(CHUNKS)
# chunks >= this index use the small dedicated tile tags
TAPER = int(os.environ.get("KTAPER", "7"))
IO_BUFS = int(os.environ.get("KIO_BUFS", "2"))
WK_BUFS = int(os.environ.get("KWK_BUFS", "2"))
TL_BUFS = int(os.environ.get("KTL_BUFS", "2"))
TP_BUFS = int(os.environ.get("KTP_BUFS", "3"))
DX_BUFS = int(os.environ.get("KDX_BUFS", "3"))
LOOKAHEAD = int(os.environ.get("KLOOKAHEAD", "3"))

F32 = mb.dt.float32
BF16 = mb.dt.bfloat16
ADD = mb.AluOpType.add


def _split_multiwaits(nc):
    """This walrus build accepts only one sync-wait per instruction.
    Hoist extra waits onto standalone single-wait InstEventSemaphore
    instructions inserted just before, on the same engine (semantically
    identical: the engine queue blocks on each in sequence)."""
    n_split = 0
    for f in nc.m.functions:
        for bb in f.blocks:
            insts = bb.instructions
            out = []
            changed = False
            for ins in insts:
                si = getattr(ins, "sync_info", None)
                if si is not None and si.on_wait is not None and len(si.on_wait) > 1:
                    waits = list(si.on_wait)
                    for k, w in enumerate(waits[:-1]):
                        ev = mb.InstEventSemaphore(
                            name=f"{ins.name}-wsplit{k}",
                            engine=ins.engine,
                            ins=[],
                            outs=[],
                            sync_info=mb.SyncInfo(on_wait=[w], on_update=[]),
                        )
                        out.append(ev)
                        n_split += 1
                    si.on_wait.clear()
                    si.on_wait.append(waits[-1])
                    changed = True
                out.append(ins)
            if changed:
                bb.instructions = out
    return n_split


def _build():
    nc = bass.Bass(trn_type="TRN2", name="triplet_loss")
    a = nc.dram_tensor("a", [S, D], F32, kind="ExternalInput")
    p = nc.dram_tensor("p", [S, D], F32, kind="ExternalInput")
    n = nc.dram_tensor("n", [S, D], F32, kind="ExternalInput")
    td = nc.dram_tensor("td", [S, 2], F32, kind="ExternalInput")
    out = nc.dram_tensor("out", [P, NCH * 2], F32, kind="ExternalOutput")

    # Partition p owns rows [p*RPP, (p+1)*RPP): contiguous per-partition
    # DRAM runs -> ideal DMA descriptors.
    av = a.rearrange("(p n) d -> p (n d)", p=P)    # [128, RPP*D]
    pv = p.rearrange("(p n) d -> p (n d)", p=P)
    nv = n.rearrange("(p n) d -> p (n d)", p=P)
    tdv = td.rearrange("(p n) t -> p n t", p=P)    # [128, RPP, 2]

    offs = [sum(CHUNKS[:i]) for i in range(NCH)]

    with tile.TileContext(nc) as tc:
        with tc.tile_pool(name="io", bufs=IO_BUFS) as io, \
             tc.tile_pool(name="wk", bufs=WK_BUFS) as wk, \
             tc.tile_pool(name="tl", bufs=TL_BUFS) as tl, \
             tc.tile_pool(name="tp", bufs=TP_BUFS) as tp, \
             tc.tile_pool(name="dx", bufs=DX_BUFS) as dx, \
             tc.tile_pool(name="res", bufs=1) as res:
            acc = res.tile([P, NCH, 2], F32)

            ap_t = [None] * NCH   # holds a, then accum-subtracted p
            an_t = [None] * NCH   # copy of a, then accum-subtracted n

            def load_a_dma(c):
                C = CHUNKS[c]
                sl = slice(offs[c] * D, (offs[c] + C) * D)
                pool, tag = (tp, "t") if c >= TAPER else (io, "p")
                at = pool.tile([P, C * D], BF16, tag=tag)
                # 2048-elem descriptor cap: the SDMA CCE/convert datapath
                # handles at most 2048 elements per descriptor
                nc.gpsimd.dma_start(out=at, in_=av[:, sl], max_dma_last_dim=2048)
                ap_t[c] = at

            def dup_a(c):
                pool, tag = (tp, "u") if c >= TAPER else (io, "n")
                a2 = pool.tile([P, CHUNKS[c] * D], BF16, tag=tag)
                nc.vector.tensor_copy(out=a2, in_=ap_t[c])     # DVE 4x dup
                an_t[c] = a2

            def load_a(c):
                load_a_dma(c)
                dup_a(c)

            def load_pn(c):
                C = CHUNKS[c]
                base = offs[c] * D
                total = C * D
                # accumulate during the cast DMA: tile := x + tile, where
                # the tile holds -a (negated on the host), giving x - a;
                # the CCE only supports ADD, and squaring kills the sign.
                # The CCE datapath handles at most 2048 elements per
                # descriptor, so issue the accumulating DMAs in 2048-elem
                # slices (same total transfer time, a few extra gens).
                for dst, src in ((ap_t[c], pv), (an_t[c], nv)):
                    for off in range(0, total, 2048):
                        w = min(2048, total - off)
                        nc.gpsimd.dma_start(
                            out=dst[:, off:off + w],
                            in_=src[:, base + off:base + off + w],
                            accum_op=ADD,
                        )

            n2_t = {}

            def chain(ci, key, ot):
                """DVE fold/reduce of a squared diff tile -> n2[ci][key]."""
                C = CHUNKS[ci]
                ot3 = ot.rearrange("p (c d) -> p c d", d=D)
                n2 = tl.tile([P, C], F32, tag="n2" + key)
                if C <= 8:
                    nc.vector.reduce_sum(out=n2, in_=ot3, axis=mb.AxisListType.X)
                else:
                    h1 = wk.tile([P, C, D // 2], BF16, tag="h1" + key)
                    nc.vector.tensor_add(
                        out=h1, in0=ot3[:, :, 0:D // 2], in1=ot3[:, :, D // 2:D]
                    )
                    w = D // 2
                    while w > 8:
                        h = w // 2
                        nc.vector.tensor_add(
                            out=h1[:, :, 0:h], in0=h1[:, :, 0:h], in1=h1[:, :, h:w]
                        )
                        w = h
                    nc.vector.reduce_sum(
                        out=n2, in_=h1[:, :, 0:8], axis=mb.AxisListType.X
                    )
                n2_t.setdefault(ci, {})[key] = n2

            def front_n(ci, copy_of=None):
                """dex (ACT), n-pair square (ACT) + fold chain (DVE); the
                chunk-(ci+3) a-copy rides behind the chain on DVE."""
                C = CHUNKS[ci]
                r0 = offs[ci]
                dex = dx.tile([P, C, 2], F32, tag="dex")
                nc.scalar.activation(out=dex, in_=td_t[:, r0:r0 + C, :],
                                     func=mb.ActivationFunctionType.Exp, scale=-1.0)
                n2_t.setdefault(ci, {})["dex"] = dex
                ot = an_t[ci]
                nc.scalar.activation(
                    out=ot, in_=ot, func=mb.ActivationFunctionType.Square
                )
                chain(ci, "n", ot)
                if copy_of is not None:
                    dup_a(copy_of)

            def sq_chain_p(ci):
                """p-pair square (lagged one chunk so its accum-DMA has
                already landed), alternating DVE/ACT by parity to balance
                engine load, then the DVE fold chain."""
                ot = ap_t[ci]
                if ci % 2 == 0 or ci == NCH - 1:
                    nc.vector.tensor_mul(out=ot, in0=ot, in1=ot)
                else:
                    nc.scalar.activation(
                        out=ot, in_=ot, func=mb.ActivationFunctionType.Square
                    )
                chain(ci, "p", ot)

            def tail(ci):
                """transcendentals (ACT) + weighted-error ops (DVE), two
                chunks behind, so no engine stalls on another."""
                dex = n2_t[ci]["dex"]
                for i, key in enumerate(("p", "n")):
                    n2 = n2_t[ci][key]
                    nc.scalar.activation(out=n2, in_=n2,
                                         func=mb.ActivationFunctionType.Sqrt)
                    nc.scalar.activation(out=n2, in_=n2,
                                         func=mb.ActivationFunctionType.Exp,
                                         scale=-1.0)
                    dcol = dex[:, :, i]
                    nc.vector.tensor_sub(out=n2, in0=dcol, in1=n2)
                    m_ = tl.tile([P, CHUNKS[ci]], F32, tag="m" + key)
                    nc.vector.tensor_mul(out=m_, in0=dcol, in1=n2)
                    nc.vector.tensor_mul(out=n2, in0=m_, in1=n2)
                    nc.vector.reduce_sum(
                        out=acc[:, ci, i:i + 1], in_=n2, axis=mb.AxisListType.X
                    )

            # software pipeline: a-loads three chunks ahead, p-pair
            # square/chain one behind, transcendental/loss tail two behind.
            load_a_dma(0)
            td_t = res.tile([P, RPP, 2], F32)
            nc.sync.dma_start(out=td_t, in_=tdv)
            dup_a(0)
            for k in range(1, LOOKAHEAD):
                load_a(k)
            for c in range(NCH):
                load_pn(c)
                if c + LOOKAHEAD < NCH:
                    load_a_dma(c + LOOKAHEAD)
                if c >= 1:
                    sq_chain_p(c - 1)
                front_n(c, copy_of=c + LOOKAHEAD if c + LOOKAHEAD < NCH else None)
                if c >= 2:
                    tail(c - 2)
            sq_chain_p(NCH - 1)
            tail(NCH - 2)
            tail(NCH - 1)

            nc.gpsimd.dma_start(out=out[:, :], in_=acc.rearrange("p c i -> p (c i)"))

    _split_multiwaits(nc)
    return nc


_CACHE = {}


def _get_nc():
    if "nc" not in _CACHE:
        _CACHE["nc"] = _build()
    return _CACHE["nc"]


def _run(inputs, **spmd_kwargs):
    # negated anchor: the DMA-accumulate path only has ADD, so the
    # kernel streams -a and accumulates p/n onto it (sign dies in the square)
    a = -np.asarray(inputs["embedding_a"], dtype=np.float32)
    p = np.asarray(inputs["embedding_p"], dtype=np.float32)
    n = np.asarray(inputs["embedding_n"], dtype=np.float32)
    tdis = np.asarray(inputs["triplets_dis"], dtype=np.float32)
    bidx = np.asarray(inputs["batch_index"])
    td = np.ascontiguousarray(tdis[bidx])

    in_maps = [
        {
            "a": a[i * S : (i + 1) * S],
            "p": p[i * S : (i + 1) * S],
            "n": n[i * S : (i + 1) * S],
            "td": td[i * S : (i + 1) * S],
        }
        for i in range(M)
    ]
    r = run_bass_kernel_spmd(_get_nc(), in_maps, core_ids=list(range(M)), **spmd_kwargs)
    total = sum(res["out"].astype(np.float64).sum() for res in r.results)
    return np.float32(total / B), r


def kernel(**inputs):
    loss, _ = _run(inputs)
    return loss
